# revision 28
# baseline (speedup 1.0000x reference)
"""ButterflyLinear Trainium2 kernel.

Math: out[b, s, i] = (sum_o x[b, s, o] * W[o, i]) * mask[s, i], with
mask[s, i] = 1 iff 4s <= i < 4s+4 (stride-4 band). Only the 4-wide band
is computed; the host scatters it into the zero-filled full output.

Sharding (8 cores): core t owns s-block t = s in [128t, 128t+128) for
all 16 batches (output columns [512t, 512t+512); no inter-core comm).

Per-core structure: s_rel = 32h + r (h: 4 PSUM banks, r in [0,32)),
o = 128c + p (c: 8 contraction chunks, p: partition). For each (c, h)
ONE matmul: stationary = W[128c:128c+128, 512t+128h : +128] (128x128
fp16), moving = x pack [p, (b, r)] (128 x 512, fp8 e3m4), accumulated
over c in fp32 PSUM bank h: bank_h[w, 32b+r] = y[b, 128t+32h+r,
512t+128h+w]. 32 matmuls of N=512.

x travels as fp8 e3m4 (4 mantissa bits): halves the dominant DMA
stream (4MB -> 2MB per core); band rel err ~1.3e-2 vs the 2e-2 gate
(quantization happens on host, so the error is deterministic). W stays
fp16 (mixed-dtype matmul; fp8 W on top would put err at ~2e-2).

Schedule: the DMA stream is ordered so W quads land just before the x
chunks that need them, with big (512KB, 4KB-row) transfers through the
body (DMA-instruction issue costs ~0.6us each and small-row transfers
run at a fraction of the ~360-400 GB/s wire rate) and single-chunk
transfers at the tail so the last matmuls start as early as possible.
A dummy-matmul train keeps TensorE busy from t~0 until the first chunk
lands so the HAM clock gate reaches 2.4 GHz before the real matmuls,
and short dummy bridges at chunk-pair boundaries stop mid-kernel
re-throttling. Banks drain in last-matmul order (Vector: banks 0,2;
Scalar: banks 1,3 - single producer engine per out tile, since the
wait-splitting pass can drop one of two cross-engine waits on a DMA);
Sync ships Vector's pair, Scalar its own in program order.

Host extracts the band: band[b, 128t+32h+r, j] = out_t[4r+j, h, 32b+r].
"""

import os
import sys
from contextlib import ExitStack

import numpy as np

if "/opt/trn_rl_repo" not in sys.path:
    sys.path.insert(0, "/opt/trn_rl_repo")

import concourse.bass as bass  # noqa: E402
import concourse.tile as tile  # noqa: E402
from concourse import bacc, mybir  # noqa: E402
from concourse.bass_utils import run_bass_kernel_spmd  # noqa: E402

B = 16  # batch
NT = 8  # s-blocks == cores
SB = 128  # s rows per block
NCH = 8  # o chunks
KC = 128  # o rows per chunk
NH = 4  # PSUM banks / 128-col W windows per block
R = 32  # s rows per window
U = B * R  # moving free size = 512

# Warmup train sized to end just at the EARLIEST plausible first-chunk
# arrival (~5.5us into the body): an oversized train becomes pure
# backlog that delays the real matmul chain (56ns/dummy once warm, and
# the PE queue is FIFO), which cost ~3us on slow-DMA runs. A small
# undershoot only risks a short (<1.5us) PE idle, which the HAM window
# tolerates without re-throttling.
NW512 = int(os.environ.get("BFK_NW512", "5"))  # head warmup N=512 matmuls
NW128 = int(os.environ.get("BFK_NW128", "22"))  # head warmup N=128 fillers
_DT = {"f8e3": mybir.dt.float8e3, "f8e4": mybir.dt.float8e4, "f16": mybir.dt.float16}
X_DT = _DT[os.environ.get("BFK_XDT", "f8e3")]
W_DT = _DT[os.environ.get("BFK_WDT", "f16")]
F16 = mybir.dt.float16
F32 = mybir.dt.float32

_STATE: dict = {}


def _build():
    if "nc" in _STATE:
        return _STATE["nc"]

    nc = bacc.Bacc("TRN2", target_bir_lowering=False, debug=False, num_devices=NT)
    # Partition-major DRAM layouts: one descriptor row per partition.
    xt = nc.dram_tensor("xt", [KC, NCH, NH, B, R], X_DT, kind="ExternalInput").ap()
    wt = nc.dram_tensor("wt", [KC, NCH, NH, KC], W_DT, kind="ExternalInput").ap()
    out = nc.dram_tensor("out", [KC, NH, U], F16, kind="ExternalOutput").ap()

    with tile.TileContext(nc) as tc, ExitStack() as ctx:
        wp = ctx.enter_context(tc.tile_pool(name="w", bufs=1))
        xp = ctx.enter_context(tc.tile_pool(name="x", bufs=1))
        pp = ctx.enter_context(tc.tile_pool(name="ps", bufs=1, space="PSUM"))
        op = ctx.enter_context(tc.tile_pool(name="o", bufs=1))

        # Warmup scratch: zeroed SBUF tile + scratch PSUM bank.
        wsc = op.tile([KC, U], F16, tag="warm")
        nc.gpsimd.memset(wsc[:], 0.0)
        psw = pp.tile([SB, U], F32, tag="psw")

        w_all = wp.tile([KC, NCH, NH, KC], W_DT, tag="w")
        x01 = xp.tile([KC, 2, NH, B, R], X_DT, tag="x01")
        x23 = xp.tile([KC, 2, NH, B, R], X_DT, tag="x23")
        x45 = xp.tile([KC, 2, NH, B, R], X_DT, tag="x45")
        x6 = xp.tile([KC, NH, B, R], X_DT, tag="x6")
        x7 = xp.tile([KC, NH, B, R], X_DT, tag="x7")

        # Stream order = consumption order; big (512KB, 4KB-row)
        # transfers keep the wire rate high. W rides the Sync HWDGE
        # ring, ALL x chunks ride the second (Scalar/ACT) HWDGE ring:
        # the SDMA engines round-robin both rings' packets, so neither
        # ring queues deep and every chunk's gating semaphore fires
        # ~0.8us after its last byte instead of ~2.7us on a deep single
        # ring (mid-stream sem lag was stalling the matmul chain). W
        # only gates chunks 0 and 4, where the chain has slack.
        nc.sync.dma_start(out=w_all[:, 0:4], in_=wt[:, 0:4])
        nc.sync.dma_start(out=w_all[:, 4:8], in_=wt[:, 4:8])
        nc.scalar.dma_start(out=x01[:], in_=xt[:, 0:2])
        nc.scalar.dma_start(out=x23[:], in_=xt[:, 2:4])
        nc.scalar.dma_start(out=x45[:], in_=xt[:, 4:6])
        nc.scalar.dma_start(out=x6[:], in_=xt[:, 6])
        nc.scalar.dma_start(out=x7[:], in_=xt[:, 7])

        # Head warmup: the first chunk can't be ready before ~5us of
        # stream time, so a dummy-matmul train keeps the PE HAM window
        # busy until then. A few N=512 to get past the cold window, then
        # N=128 fillers that self-pace (slow while cold, 81ns once warm)
        # so the overshoot past real-data arrival is tiny.
        for i in range(NW512):
            nc.tensor.matmul(psw[:], wsc[:, :KC], wsc[:], start=True, stop=True)
        for i in range(NW128):
            nc.tensor.matmul(
                psw[:, :KC], wsc[:, :KC], wsc[:, :KC], start=True, stop=True
            )

        xsrc = {
            0: lambda h: x01[:, 0, h],
            1: lambda h: x01[:, 1, h],
            2: lambda h: x23[:, 0, h],
            3: lambda h: x23[:, 1, h],
            4: lambda h: x45[:, 0, h],
            5: lambda h: x45[:, 1, h],
            6: lambda h: x6[:, h],
            7: lambda h: x7[:, h],
        }
        ps = [
            pp.tile([SB, U], F32, tag=f"ps{h}", name=f"ps_{h}") for h in range(NH)
        ]
        for c in range(NCH):
            for h in range(NH):
                nc.tensor.matmul(
                    ps[h][:],
                    w_all[:, c, h, :],
                    xsrc[c](h),
                    start=(c == 0),
                    stop=(c == NCH - 1),
                )
            if c in (1, 3, 5):
                # Bridge the expected DMA-wait gap at pair boundaries so
                # the HAM clock gate never re-throttles mid-kernel.
                for i in range(2 if c == 3 else 1):
                    nc.tensor.matmul(psw[:], wsc[:, :KC], wsc[:], start=True, stop=True)

        # Drain banks in the order the last chunk's matmuls finish:
        # Vector takes banks 0,2 and Scalar banks 1,3 so both engines
        # start as early as possible. Each out tile has a SINGLE producer
        # engine: a two-producer tile needs two waits on the out DMA and
        # the wait-splitting pass can drop the cross-engine one (observed
        # as an intermittent garbage race). Sync ships Vector's banks,
        # Scalar its own in program order.
        # DRAM out position -> bank: [0, 2, 1, 3] (host unpermutes).
        otv = op.tile([KC, 2, U], F16, tag="otv")
        ots = op.tile([KC, 2, U], F16, tag="ots")
        nc.vector.tensor_copy(otv[:, 0], ps[0][:])
        nc.scalar.copy(ots[:, 0], ps[1][:])
        nc.vector.tensor_copy(otv[:, 1], ps[2][:])
        nc.scalar.copy(ots[:, 1], ps[3][:])
        nc.sync.dma_start(out=out[:, 0:2], in_=otv[:])
        nc.scalar.dma_start(out=out[:, 2:4], in_=ots[:])

    nc.compile()
    _STATE["nc"] = nc
    return nc


def _shard(x, W):
    x_np = mybir.dt.np(X_DT)
    w_np = mybir.dt.np(W_DT)
    x = np.ascontiguousarray(np.asarray(x, dtype=np.float32)).astype(x_np)
    W = np.ascontiguousarray(np.asarray(W, dtype=np.float32)).astype(w_np)
    # xt[t][p, c, h, b, r] = x[b, 128t + 32h + r, 128c + p]
    xr = x.reshape(B, NT, NH, R, NCH, KC)  # [b, t, h, r, c, p]
    xts = np.ascontiguousarray(np.transpose(xr, (1, 5, 4, 2, 0, 3)))
    # wt[t][p, c, h, w] = W[128c + p, 512t + 128h + w]
    wr = W.reshape(NCH, KC, NT, NH, KC)  # [c, p, t, h, w]
    wts = np.ascontiguousarray(np.transpose(wr, (2, 1, 0, 3, 4)))
    return [{"xt": xts[t], "wt": wts[t]} for t in range(NT)]


def kernel(x, W, _trace=False, _trace_kwargs=None):
    nc = _build()
    in_maps = _shard(x, W)
    res = run_bass_kernel_spmd(
        nc,
        in_maps,
        list(range(NT)),
        trace=_trace,
        **(_trace_kwargs or {}),
    )
    _STATE["last_run"] = res
    band = np.empty((B, NT * SB, 4), dtype=np.float32)
    for t in range(NT):
        blk = res.results[t]["out"].astype(np.float32)  # (128, NH, U)
        blk = blk[:, [0, 2, 1, 3], :]  # DRAM position -> bank order
        v = blk.reshape(R, 4, NH, B, R)  # [r', j, h, b, r]; band at r' == r
        band[:, t * SB : (t + 1) * SB, :] = np.einsum("rjhbr->bhrj", v).reshape(
            B, SB, 4
        )
    s_idx = np.arange(NT * SB)
    y = np.zeros((B, NT * SB, NT * SB, 4), dtype=np.float32)
    y[:, s_idx, s_idx, :] = band
    return y.reshape(B, NT * SB, NT * SB * 4)
